# revision 17
# baseline (speedup 1.0000x reference)
"""Trainium2 Bass kernel for nn_MultiHeadAttention_3796751090171 (sparse_attention).

Head-parallel SPMD across 8 NeuronCores: core c computes head c's attention,
then the cores AllGather the (normalized) per-head context vectors and each
core computes a 64-column slice of the output projection — so no cross-core
reduction is ever needed (output = pure concatenation).

Math per head h (core c = h):
  Q = q_feat @ Wq[:, h*64:(h+1)*64] + bq_h          [N, 64]   (pre-scaled 1/8)
  K = k_feat @ Wk_h + bk_h                           [N, 64]
  V = v_feat @ Wv_h + bv_h                           [N, 64]
  S = Q @ K.T + pos_enc[h]   (block-diagonal only)   sparse [N, N]
  P = exp(S); Z = row sums (via ones-column in the V matmul)
  hT = (V|1).T @ expT ; hT /= Z
  -- AllGather hT over heads -> hTf [512, N] --
  outT_c = Wo[:, c*64:(c+1)*64].T @ hTf + bo_c       [64, N]
Host: out[:, c*64:(c+1)*64] = outT_c.T

Sparsity: q_batch/k_batch are SORTED, so the cross-batch mask is block-diagonal
over (q-range x k-range) batch blocks; we only compute those blocks and never
need elementwise masking (k-chunks are batch-aligned).

No max-subtraction in softmax: scores are O(10) so exp is safe in fp32; fully
masked blocks are simply never computed (prob contribution exactly 0, matching
the reference's exp(-1e9 - max) -> 0).
"""

import functools
import math

import numpy as np
import ml_dtypes

import concourse.bass as bass
import concourse.tile as tile
from concourse import bacc, mybir
from concourse.bass_utils import run_bass_kernel_spmd
from concourse.masks import make_identity

N = 3072
QD = 512
OD = 512
H = 8
D = 64
B = 8
NCORES = 8
SCALE = math.sqrt(D)

F32 = mybir.dt.float32
BF16 = mybir.dt.bfloat16
BF16_NP = ml_dtypes.bfloat16

# test.py can flip these to get a profile
TRACE = False
LAST_RESULTS = None


def _plan(q_batch, k_batch):
    """Batch block boundaries from the sorted batch-id vectors."""
    qb = np.asarray(q_batch).astype(np.int64)
    kb = np.asarray(k_batch).astype(np.int64)
    qbound = np.searchsorted(qb, np.arange(B + 1))
    kbound = np.searchsorted(kb, np.arange(B + 1))
    batches = []
    degenerate = False
    for b in range(B):
        q0, q1 = int(qbound[b]), int(qbound[b + 1])
        k0, k1 = int(kbound[b]), int(kbound[b + 1])
        if q1 > q0 and k1 > k0:
            batches.append((q0, q1, k0, k1))
        elif q1 > q0 and k1 == k0:
            # rows with no visible keys: reference gives uniform attention over
            # ALL keys; handled by numpy fallback (never happens in practice)
            degenerate = True
    return tuple(batches), degenerate


def _chunks(lo, hi, step):
    return [(o, min(step, hi - o)) for o in range(lo, hi, step)]


@functools.lru_cache(maxsize=8)
def _build(batches, has_bq, has_bk, has_bv, has_bo):
    nc = bacc.Bacc("TRN2", target_bir_lowering=False, debug=False,
                   num_devices=NCORES)

    # ---- DRAM parameters (per-core values supplied via in_maps) ----
    qfT_d = nc.dram_tensor("qfT", [QD, N], BF16, kind="ExternalInput")
    kfT_d = nc.dram_tensor("kfT", [QD, N], BF16, kind="ExternalInput")
    vfT_d = nc.dram_tensor("vfT", [QD, N], BF16, kind="ExternalInput")
    posT_d = nc.dram_tensor("posT", [N, N], BF16, kind="ExternalInput")
    wq_d = nc.dram_tensor("wq", [QD, D], BF16, kind="ExternalInput")
    wk_d = nc.dram_tensor("wk", [QD, D], BF16, kind="ExternalInput")
    wv_d = nc.dram_tensor("wv", [QD, D], BF16, kind="ExternalInput")
    woc_d = nc.dram_tensor("woc", [OD, D], BF16, kind="ExternalInput")
    bq_d = nc.dram_tensor("bq", [1, D], BF16, kind="ExternalInput") if has_bq else None
    bk_d = nc.dram_tensor("bk", [1, D], BF16, kind="ExternalInput") if has_bk else None
    bv_d = nc.dram_tensor("bv", [1, D], BF16, kind="ExternalInput") if has_bv else None
    boc_d = nc.dram_tensor("boc", [D, 1], F32, kind="ExternalInput") if has_bo else None
    out_d = nc.dram_tensor("out", [D, N], F32, kind="ExternalOutput")

    # global k-chunk list, aligned to batch boundaries (never crosses one)
    kchunk_list = []   # (koff, klen)
    batch_kchunks = []  # per batch: list of global chunk indices
    for (q0, q1, k0, k1) in batches:
        idxs = []
        for (koff, klen) in _chunks(k0, k1, 128):
            idxs.append(len(kchunk_list))
            kchunk_list.append((koff, klen))
        batch_kchunks.append(idxs)
    nch = len(kchunk_list)

    KT_T = 4  # 512 contraction split in 4 k-tiles of 128
    NQC = N // 512
    NZC = N // 128  # 24 z chunks

    with tile.TileContext(nc) as tc:
        with (
            tc.tile_pool(name="consts", bufs=1) as consts,
            tc.tile_pool(name="feat", bufs=2) as featp,
            tc.tile_pool(name="persist", bufs=1) as pers,
            tc.tile_pool(name="pos", bufs=10) as posp,
            tc.tile_pool(name="expp", bufs=6) as expp,
            tc.tile_pool(name="outp", bufs=3) as outp,
            tc.tile_pool(name="ps_s", bufs=3, space="PSUM") as ps_s,
            tc.tile_pool(name="ps_h", bufs=2, space="PSUM") as ps_h,
            tc.tile_pool(name="ps_p", bufs=3, space="PSUM") as ps_p,
            tc.tile_pool(name="dram", bufs=1, space="DRAM") as dramp,
        ):
            # ---------------- constants ----------------
            ones = consts.tile([1, N], BF16)
            nc.vector.memset(ones, 1.0)
            ident64 = consts.tile([D, D], F32)
            make_identity(nc, ident64)
            ident128 = consts.tile([128, 128], BF16)
            make_identity(nc, ident128)

            wq_sb = consts.tile([128, KT_T, D], BF16)
            wk_sb = consts.tile([128, KT_T, D], BF16)
            wv_sb = consts.tile([128, KT_T, D], BF16)
            woc_sb = consts.tile([128, KT_T, D], BF16)
            for t_d, t_sb in ((wq_d, wq_sb), (wk_d, wk_sb), (wv_d, wv_sb),
                              (woc_d, woc_sb)):
                nc.sync.dma_start(out=t_sb,
                                  in_=t_d.ap().rearrange("(t p) d -> p t d", p=128))
            bias_sb = {}
            for nm, dd in (("bq", bq_d), ("bk", bk_d), ("bv", bv_d)):
                if dd is not None:
                    t = consts.tile([1, D], BF16, tag=f"bias_{nm}")
                    nc.sync.dma_start(out=t, in_=dd[:, :])
                    bias_sb[nm] = t
            if boc_d is not None:
                boc_sb = consts.tile([D, 1], F32)
                nc.sync.dma_start(out=boc_sb, in_=boc_d[:, :])

            # persistent intermediates
            QT_sb = pers.tile([D, N], BF16)    # Q^T/8 with bias folded
            KT_sb = pers.tile([D, N], BF16)
            VT_sb = pers.tile([D, N], F32)
            V_sb = pers.tile([128, nch, D + 1], BF16)  # [k, chunk, d | ones]
            hT_sb = pers.tile([D, N], BF16)    # unnormalized h^T
            hTn_sb = pers.tile([D, N], BF16)   # normalized h^T
            Zrow_sb = pers.tile([1, N], F32)   # softmax denominators (q-major)
            zbc_sb = pers.tile([D, N], F32)    # recipZ broadcast along d

            # ---------------- projections ----------------
            def project_T(feat_d, w_sb, bias, dst):
                # dst[d, q] = (w.T @ featT)[d, q] (+ bias[d] via rank-1 mm)
                f_sb = featp.tile([128, KT_T, N], BF16, tag="feat")
                for t in range(KT_T):
                    for h in range(2):
                        hsl = slice(h * (N // 2), (h + 1) * (N // 2))
                        nc.sync.dma_start(
                            out=f_sb[:, t, hsl],
                            in_=feat_d.ap().rearrange("(t p) n -> t p n", p=128)[t, :, hsl],
                        )
                for qc in range(NQC):
                    qsl = slice(qc * 512, (qc + 1) * 512)
                    psum = ps_p.tile([128, 512], F32, tag="psp")
                    for t in range(KT_T):
                        nc.tensor.matmul(psum[0:D, :], w_sb[:, t, :],
                                         f_sb[:, t, qsl],
                                         start=(t == 0),
                                         stop=(t == KT_T - 1 and bias is None))
                    if bias is not None:
                        nc.tensor.matmul(psum[0:D, :], bias, ones[:, qsl],
                                         start=False, stop=True)
                    nc.any.tensor_copy(dst[:, qsl], psum[0:D, :])

            project_T(qfT_d, wq_sb, bias_sb.get("bq"), QT_sb)
            project_T(kfT_d, wk_sb, bias_sb.get("bk"), KT_sb)
            project_T(vfT_d, wv_sb, bias_sb.get("bv"), VT_sb)

            # V into batch-aligned k-chunks ([k, d] layout) via PE transposes
            nc.vector.memset(V_sb[:, :, D], 1.0)
            for j, (koff, klen) in enumerate(kchunk_list):
                pst = ps_p.tile([128, 512], F32, tag="psp")
                nc.tensor.transpose(pst[0:klen, 0:D], VT_sb[:, koff:koff + klen],
                                    ident64[:, :])
                nc.any.tensor_copy(V_sb[0:klen, j, 0:D], pst[0:klen, 0:D])

            # ------------- post-attention pipeline, per q-slice -------------
            SLICE = 1024
            NSL = N // SLICE

            def emit_slice(s):
                lo, hi = s * SLICE, (s + 1) * SLICE
                qsl = slice(lo, hi)
                # Z reciprocal: bounce q-major Z through DRAM into [128, w] so
                # reciprocal runs on 128 lanes, then back out flat (still
                # q-major) and broadcast-read across the 64 d-partitions with
                # contiguous row reads.
                w = SLICE // 128
                zb = dramp.tile([1, SLICE], F32, tag=f"zb{s}")
                nc.sync.dma_start(out=zb[:, :], in_=Zrow_sb[0:1, qsl])
                zres = pers.tile([128, w], F32, tag=f"zres{s}")
                nc.sync.dma_start(out=zres[:, :],
                                  in_=zb[0:1, :].rearrange("p (a b) -> (p a) b", b=w))
                zrec = pers.tile([128, w], F32, tag=f"zrec{s}")
                nc.vector.reciprocal(zrec[:, :], zres[:, :])
                zr_d = dramp.tile([128, w], F32, tag=f"zrd{s}")
                nc.sync.dma_start(out=zr_d[:, :], in_=zrec[:, :])
                zr_ap = zr_d[:, :]
                zbc_src = bass.AP(tensor=zr_ap.tensor, offset=zr_ap.offset,
                                  ap=[[0, D], [1, SLICE]])
                nc.sync.dma_start(out=zbc_sb[:, qsl], in_=zbc_src)
                # normalize hT
                nc.vector.tensor_mul(hTn_sb[:, qsl], hT_sb[:, qsl],
                                     zbc_sb[:, qsl])
                # AllGather this slice of hT over heads
                ag_in = dramp.tile([D, SLICE], BF16, tag=f"agi{s}")
                nc.sync.dma_start(out=ag_in[:, :], in_=hTn_sb[:, qsl])
                ag_out = dramp.tile([OD, SLICE], BF16, tag=f"ago{s}")
                nc.gpsimd.collective_compute(
                    "AllGather",
                    mybir.AluOpType.bypass,
                    replica_groups=[list(range(NCORES))],
                    ins=[ag_in.opt()],
                    outs=[ag_out.opt()],
                )
                hTf_sb = featp.tile([128, KT_T, SLICE], BF16, tag="feat")
                for t in range(KT_T):
                    nc.sync.dma_start(
                        out=hTf_sb[:, t, :],
                        in_=ag_out[:, :].rearrange("(t p) n -> t p n", p=128)[t],
                    )
                # output projection (column slice of Wo), transposed orientation
                for qc in range(SLICE // 512):
                    osl = slice(lo + qc * 512, lo + (qc + 1) * 512)
                    psum = ps_p.tile([128, 512], F32, tag="psp")
                    for t in range(KT_T):
                        nc.tensor.matmul(psum[0:D, :], woc_sb[:, t, :],
                                         hTf_sb[:, t, qc * 512:(qc + 1) * 512],
                                         start=(t == 0), stop=(t == KT_T - 1))
                    o_sb = outp.tile([D, 512], F32, tag="osb")
                    if boc_d is not None:
                        nc.scalar.activation(o_sb[:, :], psum[0:D, :],
                                             mybir.ActivationFunctionType.Identity,
                                             bias=boc_sb[:, 0:1])
                    else:
                        nc.any.tensor_copy(o_sb[:, :], psum[0:D, :])
                    nc.sync.dma_start(out=out_d[:, osl], in_=o_sb[:, :])

            # ---------------- attention (block-diagonal) ----------------
            emitted = 0
            for bi, (q0, q1, k0, k1) in enumerate(batches):
                for (qoff, qw) in _chunks(q0, q1, 512):
                    qsl = slice(qoff, qoff + qw)
                    psum_h = ps_h.tile([D + 1, 512], F32, tag="psh")
                    idxs = batch_kchunks[bi]
                    for ii, j in enumerate(idxs):
                        koff, klen = kchunk_list[j]
                        ksl = slice(koff, koff + klen)
                        ps = ps_s.tile([128, 512], F32, tag="pss")
                        nc.tensor.matmul(ps[0:klen, 0:qw], KT_sb[:, ksl],
                                         QT_sb[:, qsl], start=True, stop=False)
                        pos = posp.tile([128, 512], BF16, tag="pos")
                        nc.sync.dma_start(out=pos[0:klen, 0:qw],
                                          in_=posT_d[ksl, qsl])
                        # add pos_enc on the PE: accumulate I.T @ pos
                        nc.tensor.matmul(ps[0:klen, 0:qw],
                                         ident128[0:klen, 0:klen],
                                         pos[0:klen, 0:qw],
                                         start=False, stop=True)
                        expt = expp.tile([128, 512], BF16, tag="expt")
                        nc.scalar.activation(expt[0:klen, 0:qw], ps[0:klen, 0:qw],
                                             mybir.ActivationFunctionType.Exp)
                        nc.tensor.matmul(psum_h[:, 0:qw], V_sb[0:klen, j, :],
                                         expt[0:klen, 0:qw],
                                         start=(ii == 0), stop=(ii == len(idxs) - 1))
                    nc.any.tensor_copy(hT_sb[:, qsl], psum_h[0:D, 0:qw])
                    nc.any.tensor_copy(Zrow_sb[:, qsl], psum_h[D:D + 1, 0:qw])
                while emitted < NSL and (emitted + 1) * SLICE <= q1:
                    emit_slice(emitted)
                    emitted += 1
            assert emitted == NSL, (emitted, NSL, batches)

    nc.compile()
    return nc


def _kernel_numpy(q_feat, k_feat, v_feat, pos_enc, Wq, bq, Wk, bk, Wv, bv,
                  Wo, bo, q_batch, k_batch):
    """Host fallback (degenerate batch layouts only) + debugging aid."""
    Q = (q_feat @ Wq + bq).reshape(N, H, D).transpose(1, 0, 2)
    K = (k_feat @ Wk + bk).reshape(N, H, D).transpose(1, 0, 2)
    V = (v_feat @ Wv + bv).reshape(N, H, D).transpose(1, 0, 2)
    scores = np.einsum("hnd,hmd->hnm", Q, K) / SCALE + pos_enc
    mask = q_batch[:, None] != k_batch[None, :]
    scores = np.where(mask[None], np.float32(-1e9), scores)
    scores = scores - scores.max(-1, keepdims=True)
    e = np.exp(scores)
    probs = e / e.sum(-1, keepdims=True)
    h = np.einsum("hnm,hmd->hnd", probs, V)
    h = h.transpose(1, 0, 2).reshape(N, OD)
    return (h @ Wo + bo).astype(np.float32)


def kernel(q_feat, k_feat, v_feat, pos_enc, Wq, bq, Wk, bk, Wv, bv, Wo, bo,
           q_batch, k_batch):
    global LAST_RESULTS
    args = dict(q_feat=np.asarray(q_feat, np.float32),
                k_feat=np.asarray(k_feat, np.float32),
                v_feat=np.asarray(v_feat, np.float32),
                pos_enc=np.asarray(pos_enc, np.float32),
                Wq=np.asarray(Wq, np.float32), bq=np.asarray(bq, np.float32),
                Wk=np.asarray(Wk, np.float32), bk=np.asarray(bk, np.float32),
                Wv=np.asarray(Wv, np.float32), bv=np.asarray(bv, np.float32),
                Wo=np.asarray(Wo, np.float32), bo=np.asarray(bo, np.float32),
                q_batch=np.asarray(q_batch), k_batch=np.asarray(k_batch))

    batches, degenerate = _plan(args["q_batch"], args["k_batch"])
    if degenerate or not batches:
        return _kernel_numpy(**args)

    has_bq = bool(np.any(args["bq"]))
    has_bk = bool(np.any(args["bk"]))
    has_bv = bool(np.any(args["bv"]))
    has_bo = bool(np.any(args["bo"]))

    nc = _build(batches, has_bq, has_bk, has_bv, has_bo)

    # ---- host-side sharding / layout prep ----
    qfT = np.ascontiguousarray(args["q_feat"].T).astype(BF16_NP)
    kfT = np.ascontiguousarray(args["k_feat"].T).astype(BF16_NP)
    vfT = np.ascontiguousarray(args["v_feat"].T).astype(BF16_NP)

    in_maps = []
    for c in range(NCORES):
        hs = slice(c * D, (c + 1) * D)
        m = {
            "qfT": qfT, "kfT": kfT, "vfT": vfT,
            "posT": np.ascontiguousarray(
                args["pos_enc"][c].astype(BF16_NP).T),
            "wq": (args["Wq"][:, hs] / SCALE).astype(BF16_NP),
            "wk": args["Wk"][:, hs].astype(BF16_NP),
            "wv": args["Wv"][:, hs].astype(BF16_NP),
            "woc": np.ascontiguousarray(args["Wo"][:, hs]).astype(BF16_NP),
        }
        if has_bq:
            m["bq"] = (args["bq"][hs] / SCALE).astype(BF16_NP).reshape(1, D)
        if has_bk:
            m["bk"] = args["bk"][hs].astype(BF16_NP).reshape(1, D)
        if has_bv:
            m["bv"] = args["bv"][hs].astype(BF16_NP).reshape(1, D)
        if has_bo:
            m["boc"] = args["bo"][hs].astype(np.float32).reshape(D, 1)
        in_maps.append(m)

    res = run_bass_kernel_spmd(nc, in_maps, core_ids=list(range(NCORES)),
                               trace=TRACE)
    LAST_RESULTS = res
    out = np.empty((N, OD), np.float32)
    for c in range(NCORES):
        out[:, c * D:(c + 1) * D] = res.results[c]["out"].T
    return out


# revision 21
# speedup vs baseline: 1.0304x; 1.0304x over previous
"""Trainium2 Bass kernel for nn_MultiHeadAttention_3796751090171 (sparse_attention).

Head-parallel SPMD across 8 NeuronCores: core c computes head c's attention,
then the cores AllGather the (normalized) per-head context vectors and each
core computes a 64-column slice of the output projection — so no cross-core
reduction is ever needed (output = pure concatenation).

Math per head h (core c = h):
  Q = q_feat @ Wq[:, h*64:(h+1)*64] + bq_h          [N, 64]   (pre-scaled 1/8)
  K = k_feat @ Wk_h + bk_h                           [N, 64]
  V = v_feat @ Wv_h + bv_h                           [N, 64]
  S = Q @ K.T + pos_enc[h]   (block-diagonal only)   sparse [N, N]
  P = exp(S); Z = row sums (via ones-column in the V matmul)
  hT = (V|1).T @ expT ; hT /= Z
  -- AllGather hT over heads -> hTf [512, N] --
  outT_c = Wo[:, c*64:(c+1)*64].T @ hTf + bo_c       [64, N]
Host: out[:, c*64:(c+1)*64] = outT_c.T

Sparsity: q_batch/k_batch are SORTED, so the cross-batch mask is block-diagonal
over (q-range x k-range) batch blocks; we only compute those blocks and never
need elementwise masking (k-chunks are batch-aligned).

No max-subtraction in softmax: scores are O(10) so exp is safe in fp32; fully
masked blocks are simply never computed (prob contribution exactly 0, matching
the reference's exp(-1e9 - max) -> 0).
"""

import functools
import math

import numpy as np
import ml_dtypes

import concourse.bass as bass
import concourse.tile as tile
from concourse import bacc, mybir
from concourse.bass_utils import run_bass_kernel_spmd
from concourse.masks import make_identity

N = 3072
QD = 512
OD = 512
H = 8
D = 64
B = 8
NCORES = 8
SCALE = math.sqrt(D)

F32 = mybir.dt.float32
BF16 = mybir.dt.bfloat16
BF16_NP = ml_dtypes.bfloat16

# test.py can flip these to get a profile
TRACE = False
LAST_RESULTS = None


def _plan(q_batch, k_batch):
    """Batch block boundaries from the sorted batch-id vectors."""
    qb = np.asarray(q_batch).astype(np.int64)
    kb = np.asarray(k_batch).astype(np.int64)
    qbound = np.searchsorted(qb, np.arange(B + 1))
    kbound = np.searchsorted(kb, np.arange(B + 1))
    batches = []
    degenerate = False
    for b in range(B):
        q0, q1 = int(qbound[b]), int(qbound[b + 1])
        k0, k1 = int(kbound[b]), int(kbound[b + 1])
        if q1 > q0 and k1 > k0:
            batches.append((q0, q1, k0, k1))
        elif q1 > q0 and k1 == k0:
            # rows with no visible keys: reference gives uniform attention over
            # ALL keys; handled by numpy fallback (never happens in practice)
            degenerate = True
    return tuple(batches), degenerate


def _chunks(lo, hi, step):
    return [(o, min(step, hi - o)) for o in range(lo, hi, step)]


@functools.lru_cache(maxsize=8)
def _build(batches, has_bq, has_bk, has_bv, has_bo):
    nc = bacc.Bacc("TRN2", target_bir_lowering=False, debug=False,
                   num_devices=NCORES)

    # ---- DRAM parameters (per-core values supplied via in_maps) ----
    qfT_d = nc.dram_tensor("qfT", [QD, N], BF16, kind="ExternalInput")
    kfT_d = nc.dram_tensor("kfT", [QD, N], BF16, kind="ExternalInput")
    vfT_d = nc.dram_tensor("vfT", [QD, N], BF16, kind="ExternalInput")
    posT_d = nc.dram_tensor("posT", [N, N], BF16, kind="ExternalInput")
    wq_d = nc.dram_tensor("wq", [QD, D], BF16, kind="ExternalInput")
    wk_d = nc.dram_tensor("wk", [QD, D], BF16, kind="ExternalInput")
    wv_d = nc.dram_tensor("wv", [QD, D], BF16, kind="ExternalInput")
    woc_d = nc.dram_tensor("woc", [OD, D], BF16, kind="ExternalInput")
    bq_d = nc.dram_tensor("bq", [1, D], BF16, kind="ExternalInput") if has_bq else None
    bk_d = nc.dram_tensor("bk", [1, D], BF16, kind="ExternalInput") if has_bk else None
    bv_d = nc.dram_tensor("bv", [1, D], BF16, kind="ExternalInput") if has_bv else None
    boc_d = nc.dram_tensor("boc", [D, 1], F32, kind="ExternalInput") if has_bo else None
    out_d = nc.dram_tensor("out", [D, N], F32, kind="ExternalOutput")

    # global k-chunk list, aligned to batch boundaries (never crosses one)
    kchunk_list = []   # (koff, klen)
    batch_kchunks = []  # per batch: list of global chunk indices
    for (q0, q1, k0, k1) in batches:
        idxs = []
        for (koff, klen) in _chunks(k0, k1, 128):
            idxs.append(len(kchunk_list))
            kchunk_list.append((koff, klen))
        batch_kchunks.append(idxs)
    nch = len(kchunk_list)

    KT_T = 4  # 512 contraction split in 4 k-tiles of 128
    NQC = N // 512
    NZC = N // 128  # 24 z chunks

    with tile.TileContext(nc) as tc:
        with (
            tc.tile_pool(name="consts", bufs=1) as consts,
            tc.tile_pool(name="feat", bufs=2) as featp,
            tc.tile_pool(name="persist", bufs=1) as pers,
            tc.tile_pool(name="pos", bufs=10) as posp,
            tc.tile_pool(name="expp", bufs=6) as expp,
            tc.tile_pool(name="outp", bufs=3) as outp,
            tc.tile_pool(name="ps_s", bufs=3, space="PSUM") as ps_s,
            tc.tile_pool(name="ps_h", bufs=2, space="PSUM") as ps_h,
            tc.tile_pool(name="ps_p", bufs=3, space="PSUM") as ps_p,
            tc.tile_pool(name="dram", bufs=1, space="DRAM") as dramp,
        ):
            # ---------------- constants ----------------
            ones = consts.tile([1, N], BF16)
            nc.vector.memset(ones, 1.0)
            ident64 = consts.tile([D, D], F32)
            make_identity(nc, ident64)
            ident128 = consts.tile([128, 128], BF16)
            make_identity(nc, ident128)

            wq_sb = consts.tile([128, KT_T, D], BF16)
            wk_sb = consts.tile([128, KT_T, D], BF16)
            wv_sb = consts.tile([128, KT_T, D], BF16)
            woc_sb = consts.tile([128, KT_T, D], BF16)
            for t_d, t_sb in ((wq_d, wq_sb), (wk_d, wk_sb), (wv_d, wv_sb),
                              (woc_d, woc_sb)):
                nc.sync.dma_start(out=t_sb,
                                  in_=t_d.ap().rearrange("(t p) d -> p t d", p=128))
            bias_sb = {}
            for nm, dd in (("bq", bq_d), ("bk", bk_d), ("bv", bv_d)):
                if dd is not None:
                    t = consts.tile([1, D], BF16, tag=f"bias_{nm}")
                    nc.sync.dma_start(out=t, in_=dd[:, :])
                    bias_sb[nm] = t
            if boc_d is not None:
                boc_sb = consts.tile([D, 1], F32)
                nc.sync.dma_start(out=boc_sb, in_=boc_d[:, :])

            # persistent intermediates
            QT_sb = pers.tile([D, N], BF16)    # Q^T/8 with bias folded
            KT_sb = pers.tile([D, N], BF16)
            VT_sb = pers.tile([D, N], F32)
            V_sb = pers.tile([128, nch, D + 1], BF16)  # [k, chunk, d | ones]
            SLICE = 1024
            NSL = N // SLICE
            # per-q-slice tiles so slice post-processing only depends on the
            # batches that actually wrote that slice (Tile dep granularity)
            hT_s = [pers.tile([D, SLICE], BF16, tag=f"hT{s}", name=f"hT{s}")
                    for s in range(NSL)]
            hTn_s = [pers.tile([D, SLICE], BF16, tag=f"hTn{s}", name=f"hTn{s}")
                     for s in range(NSL)]
            Zrow_s = [pers.tile([1, SLICE], F32, tag=f"Zr{s}", name=f"Zr{s}")
                      for s in range(NSL)]
            zbc_s = [pers.tile([D, SLICE], F32, tag=f"zbc{s}", name=f"zbc{s}")
                     for s in range(NSL)]

            # ---------------- projections ----------------
            def project_T(feat_d, w_sb, bias, dst):
                # dst[d, q] = (w.T @ featT)[d, q] (+ bias[d] via rank-1 mm)
                f_sb = featp.tile([128, KT_T, N], BF16, tag="feat")
                for t in range(KT_T):
                    for h in range(2):
                        hsl = slice(h * (N // 2), (h + 1) * (N // 2))
                        nc.sync.dma_start(
                            out=f_sb[:, t, hsl],
                            in_=feat_d.ap().rearrange("(t p) n -> t p n", p=128)[t, :, hsl],
                        )
                for qc in range(NQC):
                    qsl = slice(qc * 512, (qc + 1) * 512)
                    psum = ps_p.tile([128, 512], F32, tag="psp")
                    for t in range(KT_T):
                        nc.tensor.matmul(psum[0:D, :], w_sb[:, t, :],
                                         f_sb[:, t, qsl],
                                         start=(t == 0),
                                         stop=(t == KT_T - 1 and bias is None))
                    if bias is not None:
                        nc.tensor.matmul(psum[0:D, :], bias, ones[:, qsl],
                                         start=False, stop=True)
                    nc.any.tensor_copy(dst[:, qsl], psum[0:D, :])

            project_T(qfT_d, wq_sb, bias_sb.get("bq"), QT_sb)
            project_T(kfT_d, wk_sb, bias_sb.get("bk"), KT_sb)
            project_T(vfT_d, wv_sb, bias_sb.get("bv"), VT_sb)

            # V into batch-aligned k-chunks ([k, d] layout) via PE transposes
            nc.vector.memset(V_sb[:, :, D], 1.0)
            for j, (koff, klen) in enumerate(kchunk_list):
                pst = ps_p.tile([128, 512], F32, tag="psp")
                nc.tensor.transpose(pst[0:klen, 0:D], VT_sb[:, koff:koff + klen],
                                    ident64[:, :])
                nc.any.tensor_copy(V_sb[0:klen, j, 0:D], pst[0:klen, 0:D])

            # ------------- post-attention pipeline, per q-slice -------------
            def emit_slice(s):
                lo = s * SLICE
                # Z reciprocal: bounce q-major Z through DRAM into [128, w] so
                # reciprocal runs on 128 lanes, then back out flat (still
                # q-major) and broadcast-read across the 64 d-partitions with
                # contiguous row reads.
                w = SLICE // 128
                zb = dramp.tile([1, SLICE], F32, tag=f"zb{s}")
                nc.sync.dma_start(out=zb[:, :], in_=Zrow_s[s][0:1, :])
                zres = pers.tile([128, w], F32, tag=f"zres{s}")
                nc.sync.dma_start(out=zres[:, :],
                                  in_=zb[0:1, :].rearrange("p (a b) -> (p a) b", b=w))
                zrec = pers.tile([128, w], F32, tag=f"zrec{s}")
                nc.vector.reciprocal(zrec[:, :], zres[:, :])
                zr_d = dramp.tile([128, w], F32, tag=f"zrd{s}")
                nc.sync.dma_start(out=zr_d[:, :], in_=zrec[:, :])
                zr_ap = zr_d[:, :]
                zbc_src = bass.AP(tensor=zr_ap.tensor, offset=zr_ap.offset,
                                  ap=[[0, D], [1, SLICE]])
                nc.sync.dma_start(out=zbc_s[s][:, :], in_=zbc_src)
                # normalize hT
                nc.vector.tensor_mul(hTn_s[s][:, :], hT_s[s][:, :],
                                     zbc_s[s][:, :])
                # AllGather this slice of hT over heads
                ag_in = dramp.tile([D, SLICE], BF16, tag=f"agi{s}")
                nc.sync.dma_start(out=ag_in[:, :], in_=hTn_s[s][:, :])
                ag_out = dramp.tile([OD, SLICE], BF16, tag=f"ago{s}")
                nc.gpsimd.collective_compute(
                    "AllGather",
                    mybir.AluOpType.bypass,
                    replica_groups=[list(range(NCORES))],
                    ins=[ag_in.opt()],
                    outs=[ag_out.opt()],
                )
                hTf_sb = featp.tile([128, KT_T, SLICE], BF16, tag="feat")
                for t in range(KT_T):
                    nc.sync.dma_start(
                        out=hTf_sb[:, t, :],
                        in_=ag_out[:, :].rearrange("(t p) n -> t p n", p=128)[t],
                    )
                # output projection (column slice of Wo), transposed orientation
                for qc in range(SLICE // 512):
                    osl = slice(lo + qc * 512, lo + (qc + 1) * 512)
                    psum = ps_p.tile([128, 512], F32, tag="psp")
                    for t in range(KT_T):
                        nc.tensor.matmul(psum[0:D, :], woc_sb[:, t, :],
                                         hTf_sb[:, t, qc * 512:(qc + 1) * 512],
                                         start=(t == 0), stop=(t == KT_T - 1))
                    o_sb = outp.tile([D, 512], F32, tag="osb")
                    if boc_d is not None:
                        nc.scalar.activation(o_sb[:, :], psum[0:D, :],
                                             mybir.ActivationFunctionType.Identity,
                                             bias=boc_sb[:, 0:1])
                    else:
                        nc.any.tensor_copy(o_sb[:, :], psum[0:D, :])
                    nc.sync.dma_start(out=out_d[:, osl], in_=o_sb[:, :])

            # ---------------- attention (block-diagonal) ----------------
            emitted = 0
            for bi, (q0, q1, k0, k1) in enumerate(batches):
                for (qoff, qw) in _chunks(q0, q1, 512):
                    qsl = slice(qoff, qoff + qw)
                    psum_h = ps_h.tile([D + 1, 512], F32, tag="psh")
                    idxs = batch_kchunks[bi]
                    for ii, j in enumerate(idxs):
                        koff, klen = kchunk_list[j]
                        ksl = slice(koff, koff + klen)
                        ps = ps_s.tile([128, 512], F32, tag="pss")
                        nc.tensor.matmul(ps[0:klen, 0:qw], KT_sb[:, ksl],
                                         QT_sb[:, qsl], start=True, stop=False)
                        pos = posp.tile([128, 512], BF16, tag="pos")
                        nc.sync.dma_start(out=pos[0:klen, 0:qw],
                                          in_=posT_d[ksl, qsl])
                        # add pos_enc on the PE: accumulate I.T @ pos
                        nc.tensor.matmul(ps[0:klen, 0:qw],
                                         ident128[0:klen, 0:klen],
                                         pos[0:klen, 0:qw],
                                         start=False, stop=True)
                        expt = expp.tile([128, 512], BF16, tag="expt")
                        nc.scalar.activation(expt[0:klen, 0:qw], ps[0:klen, 0:qw],
                                             mybir.ActivationFunctionType.Exp)
                        nc.tensor.matmul(psum_h[:, 0:qw], V_sb[0:klen, j, :],
                                         expt[0:klen, 0:qw],
                                         start=(ii == 0), stop=(ii == len(idxs) - 1))
                    # copy h/Z out of PSUM, split at q-slice boundaries
                    seg = qoff
                    while seg < qoff + qw:
                        s = seg // SLICE
                        send = min(qoff + qw, (s + 1) * SLICE)
                        lsl = slice(seg - s * SLICE, send - s * SLICE)
                        psl = slice(seg - qoff, send - qoff)
                        nc.any.tensor_copy(hT_s[s][:, lsl], psum_h[0:D, psl])
                        nc.any.tensor_copy(Zrow_s[s][:, lsl],
                                           psum_h[D:D + 1, psl])
                        seg = send
                while emitted < NSL and (emitted + 1) * SLICE <= q1:
                    emit_slice(emitted)
                    emitted += 1
            assert emitted == NSL, (emitted, NSL, batches)

    nc.compile()
    return nc


def _kernel_numpy(q_feat, k_feat, v_feat, pos_enc, Wq, bq, Wk, bk, Wv, bv,
                  Wo, bo, q_batch, k_batch):
    """Host fallback (degenerate batch layouts only) + debugging aid."""
    Q = (q_feat @ Wq + bq).reshape(N, H, D).transpose(1, 0, 2)
    K = (k_feat @ Wk + bk).reshape(N, H, D).transpose(1, 0, 2)
    V = (v_feat @ Wv + bv).reshape(N, H, D).transpose(1, 0, 2)
    scores = np.einsum("hnd,hmd->hnm", Q, K) / SCALE + pos_enc
    mask = q_batch[:, None] != k_batch[None, :]
    scores = np.where(mask[None], np.float32(-1e9), scores)
    scores = scores - scores.max(-1, keepdims=True)
    e = np.exp(scores)
    probs = e / e.sum(-1, keepdims=True)
    h = np.einsum("hnm,hmd->hnd", probs, V)
    h = h.transpose(1, 0, 2).reshape(N, OD)
    return (h @ Wo + bo).astype(np.float32)


def kernel(q_feat, k_feat, v_feat, pos_enc, Wq, bq, Wk, bk, Wv, bv, Wo, bo,
           q_batch, k_batch):
    global LAST_RESULTS
    args = dict(q_feat=np.asarray(q_feat, np.float32),
                k_feat=np.asarray(k_feat, np.float32),
                v_feat=np.asarray(v_feat, np.float32),
                pos_enc=np.asarray(pos_enc, np.float32),
                Wq=np.asarray(Wq, np.float32), bq=np.asarray(bq, np.float32),
                Wk=np.asarray(Wk, np.float32), bk=np.asarray(bk, np.float32),
                Wv=np.asarray(Wv, np.float32), bv=np.asarray(bv, np.float32),
                Wo=np.asarray(Wo, np.float32), bo=np.asarray(bo, np.float32),
                q_batch=np.asarray(q_batch), k_batch=np.asarray(k_batch))

    batches, degenerate = _plan(args["q_batch"], args["k_batch"])
    if degenerate or not batches:
        return _kernel_numpy(**args)

    has_bq = bool(np.any(args["bq"]))
    has_bk = bool(np.any(args["bk"]))
    has_bv = bool(np.any(args["bv"]))
    has_bo = bool(np.any(args["bo"]))

    nc = _build(batches, has_bq, has_bk, has_bv, has_bo)

    # ---- host-side sharding / layout prep ----
    qfT = np.ascontiguousarray(args["q_feat"].T).astype(BF16_NP)
    kfT = np.ascontiguousarray(args["k_feat"].T).astype(BF16_NP)
    vfT = np.ascontiguousarray(args["v_feat"].T).astype(BF16_NP)

    in_maps = []
    for c in range(NCORES):
        hs = slice(c * D, (c + 1) * D)
        m = {
            "qfT": qfT, "kfT": kfT, "vfT": vfT,
            "posT": np.ascontiguousarray(
                args["pos_enc"][c].astype(BF16_NP).T),
            "wq": (args["Wq"][:, hs] / SCALE).astype(BF16_NP),
            "wk": args["Wk"][:, hs].astype(BF16_NP),
            "wv": args["Wv"][:, hs].astype(BF16_NP),
            "woc": np.ascontiguousarray(args["Wo"][:, hs]).astype(BF16_NP),
        }
        if has_bq:
            m["bq"] = (args["bq"][hs] / SCALE).astype(BF16_NP).reshape(1, D)
        if has_bk:
            m["bk"] = args["bk"][hs].astype(BF16_NP).reshape(1, D)
        if has_bv:
            m["bv"] = args["bv"][hs].astype(BF16_NP).reshape(1, D)
        if has_bo:
            m["boc"] = args["bo"][hs].astype(np.float32).reshape(D, 1)
        in_maps.append(m)

    res = run_bass_kernel_spmd(nc, in_maps, core_ids=list(range(NCORES)),
                               trace=TRACE)
    LAST_RESULTS = res
    out = np.empty((N, OD), np.float32)
    for c in range(NCORES):
        out[:, c * D:(c + 1) * D] = res.results[c]["out"].T
    return out


# revision 22
# speedup vs baseline: 1.0861x; 1.0540x over previous
"""Trainium2 Bass kernel for nn_MultiHeadAttention_3796751090171 (sparse_attention).

Head-parallel SPMD across 8 NeuronCores: core c computes head c's attention,
then the cores AllGather the (normalized) per-head context vectors and each
core computes a 64-column slice of the output projection — so no cross-core
reduction is ever needed (output = pure concatenation).

Math per head h (core c = h):
  Q = q_feat @ Wq[:, h*64:(h+1)*64] + bq_h          [N, 64]   (pre-scaled 1/8)
  K = k_feat @ Wk_h + bk_h                           [N, 64]
  V = v_feat @ Wv_h + bv_h                           [N, 64]
  S = Q @ K.T + pos_enc[h]   (block-diagonal only)   sparse [N, N]
  P = exp(S); Z = row sums (via ones-column in the V matmul)
  hT = (V|1).T @ expT ; hT /= Z
  -- AllGather hT over heads -> hTf [512, N] --
  outT_c = Wo[:, c*64:(c+1)*64].T @ hTf + bo_c       [64, N]
Host: out[:, c*64:(c+1)*64] = outT_c.T

Sparsity: q_batch/k_batch are SORTED, so the cross-batch mask is block-diagonal
over (q-range x k-range) batch blocks; we only compute those blocks and never
need elementwise masking (k-chunks are batch-aligned).

No max-subtraction in softmax: scores are O(10) so exp is safe in fp32; fully
masked blocks are simply never computed (prob contribution exactly 0, matching
the reference's exp(-1e9 - max) -> 0).
"""

import functools
import math

import numpy as np
import ml_dtypes

import concourse.bass as bass
import concourse.tile as tile
from concourse import bacc, mybir
from concourse.bass_utils import run_bass_kernel_spmd
from concourse.masks import make_identity

N = 3072
QD = 512
OD = 512
H = 8
D = 64
B = 8
NCORES = 8
SCALE = math.sqrt(D)

F32 = mybir.dt.float32
BF16 = mybir.dt.bfloat16
BF16_NP = ml_dtypes.bfloat16

# test.py can flip these to get a profile
TRACE = False
LAST_RESULTS = None


def _plan(q_batch, k_batch):
    """Batch block boundaries from the sorted batch-id vectors."""
    qb = np.asarray(q_batch).astype(np.int64)
    kb = np.asarray(k_batch).astype(np.int64)
    qbound = np.searchsorted(qb, np.arange(B + 1))
    kbound = np.searchsorted(kb, np.arange(B + 1))
    batches = []
    degenerate = False
    for b in range(B):
        q0, q1 = int(qbound[b]), int(qbound[b + 1])
        k0, k1 = int(kbound[b]), int(kbound[b + 1])
        if q1 > q0 and k1 > k0:
            batches.append((q0, q1, k0, k1))
        elif q1 > q0 and k1 == k0:
            # rows with no visible keys: reference gives uniform attention over
            # ALL keys; handled by numpy fallback (never happens in practice)
            degenerate = True
    return tuple(batches), degenerate


def _chunks(lo, hi, step):
    return [(o, min(step, hi - o)) for o in range(lo, hi, step)]


@functools.lru_cache(maxsize=8)
def _build(batches, has_bq, has_bk, has_bv, has_bo):
    nc = bacc.Bacc("TRN2", target_bir_lowering=False, debug=False,
                   num_devices=NCORES)

    # ---- DRAM parameters (per-core values supplied via in_maps) ----
    qfT_d = nc.dram_tensor("qfT", [QD, N], BF16, kind="ExternalInput")
    kfT_d = nc.dram_tensor("kfT", [QD, N], BF16, kind="ExternalInput")
    vfT_d = nc.dram_tensor("vfT", [QD, N], BF16, kind="ExternalInput")
    posT_d = nc.dram_tensor("posT", [N, N], BF16, kind="ExternalInput")
    wq_d = nc.dram_tensor("wq", [QD, D], BF16, kind="ExternalInput")
    wk_d = nc.dram_tensor("wk", [QD, D], BF16, kind="ExternalInput")
    wv_d = nc.dram_tensor("wv", [QD, D], BF16, kind="ExternalInput")
    woc_d = nc.dram_tensor("woc", [OD, D], BF16, kind="ExternalInput")
    bq_d = nc.dram_tensor("bq", [1, D], BF16, kind="ExternalInput") if has_bq else None
    bk_d = nc.dram_tensor("bk", [1, D], BF16, kind="ExternalInput") if has_bk else None
    bv_d = nc.dram_tensor("bv", [1, D], BF16, kind="ExternalInput") if has_bv else None
    boc_d = nc.dram_tensor("boc", [D, 1], F32, kind="ExternalInput") if has_bo else None
    out_d = nc.dram_tensor("out", [D, N], F32, kind="ExternalOutput")

    # global k-chunk list, aligned to batch boundaries (never crosses one)
    kchunk_list = []   # (koff, klen)
    batch_kchunks = []  # per batch: list of global chunk indices
    for (q0, q1, k0, k1) in batches:
        idxs = []
        for (koff, klen) in _chunks(k0, k1, 128):
            idxs.append(len(kchunk_list))
            kchunk_list.append((koff, klen))
        batch_kchunks.append(idxs)
    nch = len(kchunk_list)

    KT_T = 4  # 512 contraction split in 4 k-tiles of 128
    NQC = N // 512
    NZC = N // 128  # 24 z chunks

    with tile.TileContext(nc) as tc:
        with (
            tc.tile_pool(name="consts", bufs=1) as consts,
            tc.tile_pool(name="feat", bufs=2) as featp,
            tc.tile_pool(name="persist", bufs=1) as pers,
            tc.tile_pool(name="pos", bufs=10) as posp,
            tc.tile_pool(name="expp", bufs=6) as expp,
            tc.tile_pool(name="outp", bufs=3) as outp,
            tc.tile_pool(name="ps_s", bufs=3, space="PSUM") as ps_s,
            tc.tile_pool(name="ps_h", bufs=2, space="PSUM") as ps_h,
            tc.tile_pool(name="ps_p", bufs=3, space="PSUM") as ps_p,
            tc.tile_pool(name="dram", bufs=1, space="DRAM") as dramp,
        ):
            # ---------------- constants ----------------
            ones = consts.tile([1, N], BF16)
            nc.vector.memset(ones, 1.0)
            ident64 = consts.tile([D, D], F32)
            make_identity(nc, ident64)

            # warmup collective: pays the CC barrier/firmware init cost up
            # front, overlapped with the projection phase
            ccw_in = dramp.tile([1, 8], F32, tag="ccwi")
            ccw_out = dramp.tile([1, 64], F32, tag="ccwo")
            nc.vector.memset(ccw_sb := consts.tile([1, 8], F32, name="ccw_sb"), 0.0)
            nc.gpsimd.dma_start(out=ccw_in[:, :], in_=ccw_sb[:, :])
            nc.gpsimd.collective_compute(
                "AllGather",
                mybir.AluOpType.bypass,
                replica_groups=[list(range(NCORES))],
                ins=[ccw_in.opt()],
                outs=[ccw_out.opt()],
            )

            wq_sb = consts.tile([128, KT_T, D], BF16)
            wk_sb = consts.tile([128, KT_T, D], BF16)
            wv_sb = consts.tile([128, KT_T, D], BF16)
            woc_sb = consts.tile([128, KT_T, D], BF16)
            for t_d, t_sb in ((wq_d, wq_sb), (wk_d, wk_sb), (wv_d, wv_sb),
                              (woc_d, woc_sb)):
                nc.sync.dma_start(out=t_sb,
                                  in_=t_d.ap().rearrange("(t p) d -> p t d", p=128))
            bias_sb = {}
            for nm, dd in (("bq", bq_d), ("bk", bk_d), ("bv", bv_d)):
                if dd is not None:
                    t = consts.tile([1, D], BF16, tag=f"bias_{nm}")
                    nc.sync.dma_start(out=t, in_=dd[:, :])
                    bias_sb[nm] = t
            if boc_d is not None:
                boc_sb = consts.tile([D, 1], F32)
                nc.sync.dma_start(out=boc_sb, in_=boc_d[:, :])

            # persistent intermediates
            QT_sb = pers.tile([D, N], BF16)    # Q^T/8 with bias folded
            KT_sb = pers.tile([D, N], BF16)
            VT_sb = pers.tile([D, N], F32)
            V_sb = pers.tile([128, nch, D + 1], BF16)  # [k, chunk, d | ones]
            SLICE = 1024
            NSL = N // SLICE
            # per-q-slice tiles so slice post-processing only depends on the
            # batches that actually wrote that slice (Tile dep granularity)
            hT_s = [pers.tile([D, SLICE], BF16, tag=f"hT{s}", name=f"hT{s}")
                    for s in range(NSL)]
            hTn_s = [pers.tile([D, SLICE], BF16, tag=f"hTn{s}", name=f"hTn{s}")
                     for s in range(NSL)]
            Zrow_s = [pers.tile([1, SLICE], F32, tag=f"Zr{s}", name=f"Zr{s}")
                      for s in range(NSL)]
            zbc_s = [pers.tile([D, SLICE], F32, tag=f"zbc{s}", name=f"zbc{s}")
                     for s in range(NSL)]

            # ---------------- projections ----------------
            def project_T(feat_d, w_sb, bias, dst):
                # dst[d, q] = (w.T @ featT)[d, q] (+ bias[d] via rank-1 mm)
                f_sb = featp.tile([128, KT_T, N], BF16, tag="feat")
                for t in range(KT_T):
                    for h in range(2):
                        hsl = slice(h * (N // 2), (h + 1) * (N // 2))
                        nc.sync.dma_start(
                            out=f_sb[:, t, hsl],
                            in_=feat_d.ap().rearrange("(t p) n -> t p n", p=128)[t, :, hsl],
                        )
                for qc in range(NQC):
                    qsl = slice(qc * 512, (qc + 1) * 512)
                    psum = ps_p.tile([128, 512], F32, tag="psp")
                    for t in range(KT_T):
                        nc.tensor.matmul(psum[0:D, :], w_sb[:, t, :],
                                         f_sb[:, t, qsl],
                                         start=(t == 0),
                                         stop=(t == KT_T - 1 and bias is None))
                    if bias is not None:
                        nc.tensor.matmul(psum[0:D, :], bias, ones[:, qsl],
                                         start=False, stop=True)
                    nc.any.tensor_copy(dst[:, qsl], psum[0:D, :])

            project_T(qfT_d, wq_sb, bias_sb.get("bq"), QT_sb)
            project_T(kfT_d, wk_sb, bias_sb.get("bk"), KT_sb)
            project_T(vfT_d, wv_sb, bias_sb.get("bv"), VT_sb)

            # V into batch-aligned k-chunks ([k, d] layout) via PE transposes
            nc.vector.memset(V_sb[:, :, D], 1.0)
            for j, (koff, klen) in enumerate(kchunk_list):
                pst = ps_p.tile([128, 512], F32, tag="psp")
                nc.tensor.transpose(pst[0:klen, 0:D], VT_sb[:, koff:koff + klen],
                                    ident64[:, :])
                nc.any.tensor_copy(V_sb[0:klen, j, 0:D], pst[0:klen, 0:D])

            # ------------- post-attention pipeline, per q-slice -------------
            def emit_slice(s):
                lo = s * SLICE
                # Z reciprocal: bounce q-major Z through DRAM into [128, w] so
                # reciprocal runs on 128 lanes, then back out flat (still
                # q-major) and broadcast-read across the 64 d-partitions with
                # contiguous row reads.
                w = SLICE // 128
                zb = dramp.tile([1, SLICE], F32, tag=f"zb{s}")
                nc.gpsimd.dma_start(out=zb[:, :], in_=Zrow_s[s][0:1, :])
                zres = pers.tile([128, w], F32, tag=f"zres{s}")
                nc.gpsimd.dma_start(out=zres[:, :],
                                    in_=zb[0:1, :].rearrange("p (a b) -> (p a) b", b=w))
                zrec = pers.tile([128, w], F32, tag=f"zrec{s}")
                nc.vector.reciprocal(zrec[:, :], zres[:, :])
                zr_d = dramp.tile([128, w], F32, tag=f"zrd{s}")
                nc.gpsimd.dma_start(out=zr_d[:, :], in_=zrec[:, :])
                zr_ap = zr_d[:, :]
                zbc_src = bass.AP(tensor=zr_ap.tensor, offset=zr_ap.offset,
                                  ap=[[0, D], [1, SLICE]])
                nc.gpsimd.dma_start(out=zbc_s[s][:, :], in_=zbc_src)
                # normalize hT
                nc.vector.tensor_mul(hTn_s[s][:, :], hT_s[s][:, :],
                                     zbc_s[s][:, :])
                # AllGather this slice of hT over heads
                ag_in = dramp.tile([D, SLICE], BF16, tag=f"agi{s}")
                nc.gpsimd.dma_start(out=ag_in[:, :], in_=hTn_s[s][:, :])
                ag_out = dramp.tile([OD, SLICE], BF16, tag=f"ago{s}")
                nc.gpsimd.collective_compute(
                    "AllGather",
                    mybir.AluOpType.bypass,
                    replica_groups=[list(range(NCORES))],
                    ins=[ag_in.opt()],
                    outs=[ag_out.opt()],
                )
                hTf_sb = featp.tile([128, KT_T, SLICE], BF16, tag="feat")
                for t in range(KT_T):
                    nc.gpsimd.dma_start(
                        out=hTf_sb[:, t, :],
                        in_=ag_out[:, :].rearrange("(t p) n -> t p n", p=128)[t],
                    )
                # output projection (column slice of Wo), transposed orientation
                for qc in range(SLICE // 512):
                    osl = slice(lo + qc * 512, lo + (qc + 1) * 512)
                    psum = ps_p.tile([128, 512], F32, tag="psp")
                    for t in range(KT_T):
                        nc.tensor.matmul(psum[0:D, :], woc_sb[:, t, :],
                                         hTf_sb[:, t, qc * 512:(qc + 1) * 512],
                                         start=(t == 0), stop=(t == KT_T - 1))
                    o_sb = outp.tile([D, 512], F32, tag="osb")
                    if boc_d is not None:
                        nc.scalar.activation(o_sb[:, :], psum[0:D, :],
                                             mybir.ActivationFunctionType.Identity,
                                             bias=boc_sb[:, 0:1])
                    else:
                        nc.any.tensor_copy(o_sb[:, :], psum[0:D, :])
                    nc.gpsimd.dma_start(out=out_d[:, osl], in_=o_sb[:, :])

            # ---------------- attention (block-diagonal) ----------------
            emitted = 0
            for bi, (q0, q1, k0, k1) in enumerate(batches):
                for (qoff, qw) in _chunks(q0, q1, 512):
                    qsl = slice(qoff, qoff + qw)
                    psum_h = ps_h.tile([D + 1, 512], F32, tag="psh")
                    idxs = batch_kchunks[bi]
                    for ii, j in enumerate(idxs):
                        koff, klen = kchunk_list[j]
                        ksl = slice(koff, koff + klen)
                        ps = ps_s.tile([128, 512], F32, tag="pss")
                        nc.tensor.matmul(ps[0:klen, 0:qw], KT_sb[:, ksl],
                                         QT_sb[:, qsl], start=True, stop=True)
                        pos = posp.tile([128, 512], BF16, tag="pos")
                        nc.sync.dma_start(out=pos[0:klen, 0:qw],
                                          in_=posT_d[ksl, qsl])
                        nc.vector.tensor_add(ps[0:klen, 0:qw], ps[0:klen, 0:qw],
                                             pos[0:klen, 0:qw])
                        expt = expp.tile([128, 512], BF16, tag="expt")
                        nc.scalar.activation(expt[0:klen, 0:qw], ps[0:klen, 0:qw],
                                             mybir.ActivationFunctionType.Exp)
                        nc.tensor.matmul(psum_h[:, 0:qw], V_sb[0:klen, j, :],
                                         expt[0:klen, 0:qw],
                                         start=(ii == 0), stop=(ii == len(idxs) - 1))
                    # copy h/Z out of PSUM, split at q-slice boundaries
                    seg = qoff
                    while seg < qoff + qw:
                        s = seg // SLICE
                        send = min(qoff + qw, (s + 1) * SLICE)
                        lsl = slice(seg - s * SLICE, send - s * SLICE)
                        psl = slice(seg - qoff, send - qoff)
                        nc.any.tensor_copy(hT_s[s][:, lsl], psum_h[0:D, psl])
                        nc.any.tensor_copy(Zrow_s[s][:, lsl],
                                           psum_h[D:D + 1, psl])
                        seg = send
                while emitted < NSL and (emitted + 1) * SLICE <= q1:
                    emit_slice(emitted)
                    emitted += 1
            assert emitted == NSL, (emitted, NSL, batches)

    nc.compile()
    return nc


def _kernel_numpy(q_feat, k_feat, v_feat, pos_enc, Wq, bq, Wk, bk, Wv, bv,
                  Wo, bo, q_batch, k_batch):
    """Host fallback (degenerate batch layouts only) + debugging aid."""
    Q = (q_feat @ Wq + bq).reshape(N, H, D).transpose(1, 0, 2)
    K = (k_feat @ Wk + bk).reshape(N, H, D).transpose(1, 0, 2)
    V = (v_feat @ Wv + bv).reshape(N, H, D).transpose(1, 0, 2)
    scores = np.einsum("hnd,hmd->hnm", Q, K) / SCALE + pos_enc
    mask = q_batch[:, None] != k_batch[None, :]
    scores = np.where(mask[None], np.float32(-1e9), scores)
    scores = scores - scores.max(-1, keepdims=True)
    e = np.exp(scores)
    probs = e / e.sum(-1, keepdims=True)
    h = np.einsum("hnm,hmd->hnd", probs, V)
    h = h.transpose(1, 0, 2).reshape(N, OD)
    return (h @ Wo + bo).astype(np.float32)


def kernel(q_feat, k_feat, v_feat, pos_enc, Wq, bq, Wk, bk, Wv, bv, Wo, bo,
           q_batch, k_batch):
    global LAST_RESULTS
    args = dict(q_feat=np.asarray(q_feat, np.float32),
                k_feat=np.asarray(k_feat, np.float32),
                v_feat=np.asarray(v_feat, np.float32),
                pos_enc=np.asarray(pos_enc, np.float32),
                Wq=np.asarray(Wq, np.float32), bq=np.asarray(bq, np.float32),
                Wk=np.asarray(Wk, np.float32), bk=np.asarray(bk, np.float32),
                Wv=np.asarray(Wv, np.float32), bv=np.asarray(bv, np.float32),
                Wo=np.asarray(Wo, np.float32), bo=np.asarray(bo, np.float32),
                q_batch=np.asarray(q_batch), k_batch=np.asarray(k_batch))

    batches, degenerate = _plan(args["q_batch"], args["k_batch"])
    if degenerate or not batches:
        return _kernel_numpy(**args)

    has_bq = bool(np.any(args["bq"]))
    has_bk = bool(np.any(args["bk"]))
    has_bv = bool(np.any(args["bv"]))
    has_bo = bool(np.any(args["bo"]))

    nc = _build(batches, has_bq, has_bk, has_bv, has_bo)

    # ---- host-side sharding / layout prep ----
    qfT = np.ascontiguousarray(args["q_feat"].T).astype(BF16_NP)
    kfT = np.ascontiguousarray(args["k_feat"].T).astype(BF16_NP)
    vfT = np.ascontiguousarray(args["v_feat"].T).astype(BF16_NP)

    in_maps = []
    for c in range(NCORES):
        hs = slice(c * D, (c + 1) * D)
        m = {
            "qfT": qfT, "kfT": kfT, "vfT": vfT,
            "posT": np.ascontiguousarray(
                args["pos_enc"][c].astype(BF16_NP).T),
            "wq": (args["Wq"][:, hs] / SCALE).astype(BF16_NP),
            "wk": args["Wk"][:, hs].astype(BF16_NP),
            "wv": args["Wv"][:, hs].astype(BF16_NP),
            "woc": np.ascontiguousarray(args["Wo"][:, hs]).astype(BF16_NP),
        }
        if has_bq:
            m["bq"] = (args["bq"][hs] / SCALE).astype(BF16_NP).reshape(1, D)
        if has_bk:
            m["bk"] = args["bk"][hs].astype(BF16_NP).reshape(1, D)
        if has_bv:
            m["bv"] = args["bv"][hs].astype(BF16_NP).reshape(1, D)
        if has_bo:
            m["boc"] = args["bo"][hs].astype(np.float32).reshape(D, 1)
        in_maps.append(m)

    res = run_bass_kernel_spmd(nc, in_maps, core_ids=list(range(NCORES)),
                               trace=TRACE)
    LAST_RESULTS = res
    out = np.empty((N, OD), np.float32)
    for c in range(NCORES):
        out[:, c * D:(c + 1) * D] = res.results[c]["out"].T
    return out


# revision 23
# speedup vs baseline: 1.2209x; 1.1241x over previous
"""Trainium2 Bass kernel for nn_MultiHeadAttention_3796751090171 (sparse_attention).

Head-parallel SPMD across 8 NeuronCores: core c computes head c's attention,
then the cores AllGather the (normalized) per-head context vectors and each
core computes a 64-column slice of the output projection — so no cross-core
reduction is ever needed (output = pure concatenation).

Math per head h (core c = h):
  Q = q_feat @ Wq[:, h*64:(h+1)*64] + bq_h          [N, 64]   (pre-scaled 1/8)
  K = k_feat @ Wk_h + bk_h                           [N, 64]
  V = v_feat @ Wv_h + bv_h                           [N, 64]
  S = Q @ K.T + pos_enc[h]   (block-diagonal only)   sparse [N, N]
  P = exp(S); Z = row sums (via ones-column in the V matmul)
  hT = (V|1).T @ expT ; hT /= Z
  -- AllGather hT over heads -> hTf [512, N] --
  outT_c = Wo[:, c*64:(c+1)*64].T @ hTf + bo_c       [64, N]
Host: out[:, c*64:(c+1)*64] = outT_c.T

Sparsity: q_batch/k_batch are SORTED, so the cross-batch mask is block-diagonal
over (q-range x k-range) batch blocks; we only compute those blocks and never
need elementwise masking (k-chunks are batch-aligned).

No max-subtraction in softmax: scores are O(10) so exp is safe in fp32; fully
masked blocks are simply never computed (prob contribution exactly 0, matching
the reference's exp(-1e9 - max) -> 0).
"""

import functools
import math

import numpy as np
import ml_dtypes

import concourse.bass as bass
import concourse.tile as tile
from concourse import bacc, mybir
from concourse.bass_utils import run_bass_kernel_spmd
from concourse.masks import make_identity

N = 3072
QD = 512
OD = 512
H = 8
D = 64
B = 8
NCORES = 8
SCALE = math.sqrt(D)

F32 = mybir.dt.float32
BF16 = mybir.dt.bfloat16
BF16_NP = ml_dtypes.bfloat16

# test.py can flip these to get a profile
TRACE = False
LAST_RESULTS = None


def _plan(q_batch, k_batch):
    """Batch block boundaries from the sorted batch-id vectors."""
    qb = np.asarray(q_batch).astype(np.int64)
    kb = np.asarray(k_batch).astype(np.int64)
    qbound = np.searchsorted(qb, np.arange(B + 1))
    kbound = np.searchsorted(kb, np.arange(B + 1))
    batches = []
    degenerate = False
    for b in range(B):
        q0, q1 = int(qbound[b]), int(qbound[b + 1])
        k0, k1 = int(kbound[b]), int(kbound[b + 1])
        if q1 > q0 and k1 > k0:
            batches.append((q0, q1, k0, k1))
        elif q1 > q0 and k1 == k0:
            # rows with no visible keys: reference gives uniform attention over
            # ALL keys; handled by numpy fallback (never happens in practice)
            degenerate = True
    return tuple(batches), degenerate


def _chunks(lo, hi, step):
    return [(o, min(step, hi - o)) for o in range(lo, hi, step)]


@functools.lru_cache(maxsize=8)
def _build(batches, has_bq, has_bk, has_bv, has_bo):
    nc = bacc.Bacc("TRN2", target_bir_lowering=False, debug=False,
                   num_devices=NCORES)

    # ---- DRAM parameters (per-core values supplied via in_maps) ----
    qfT_d = nc.dram_tensor("qfT", [QD, N], BF16, kind="ExternalInput")
    kfT_d = nc.dram_tensor("kfT", [QD, N], BF16, kind="ExternalInput")
    vfT_d = nc.dram_tensor("vfT", [QD, N], BF16, kind="ExternalInput")
    posT_d = nc.dram_tensor("posT", [N, N], BF16, kind="ExternalInput")
    wq_d = nc.dram_tensor("wq", [QD, D], BF16, kind="ExternalInput")
    wk_d = nc.dram_tensor("wk", [QD, D], BF16, kind="ExternalInput")
    wv_d = nc.dram_tensor("wv", [QD, D], BF16, kind="ExternalInput")
    woc_d = nc.dram_tensor("woc", [OD, D], BF16, kind="ExternalInput")
    bq_d = nc.dram_tensor("bq", [1, D], BF16, kind="ExternalInput") if has_bq else None
    bk_d = nc.dram_tensor("bk", [1, D], BF16, kind="ExternalInput") if has_bk else None
    bv_d = nc.dram_tensor("bv", [1, D], BF16, kind="ExternalInput") if has_bv else None
    boc_d = nc.dram_tensor("boc", [D, 1], F32, kind="ExternalInput") if has_bo else None
    out_d = nc.dram_tensor("out", [D, N], F32, kind="ExternalOutput")

    # global k-chunk list, aligned to batch boundaries (never crosses one)
    kchunk_list = []   # (koff, klen)
    batch_kchunks = []  # per batch: list of global chunk indices
    for (q0, q1, k0, k1) in batches:
        idxs = []
        for (koff, klen) in _chunks(k0, k1, 128):
            idxs.append(len(kchunk_list))
            kchunk_list.append((koff, klen))
        batch_kchunks.append(idxs)
    nch = len(kchunk_list)

    KT_T = 4  # 512 contraction split in 4 k-tiles of 128
    NQC = N // 512
    NZC = N // 128  # 24 z chunks

    with tile.TileContext(nc) as tc:
        with (
            tc.tile_pool(name="consts", bufs=1) as consts,
            tc.tile_pool(name="feat", bufs=2) as featp,
            tc.tile_pool(name="persist", bufs=1) as pers,
            tc.tile_pool(name="pos", bufs=10) as posp,
            tc.tile_pool(name="expp", bufs=6) as expp,
            tc.tile_pool(name="outp", bufs=3) as outp,
            tc.tile_pool(name="ps_s", bufs=3, space="PSUM") as ps_s,
            tc.tile_pool(name="ps_h", bufs=2, space="PSUM") as ps_h,
            tc.tile_pool(name="ps_p", bufs=3, space="PSUM") as ps_p,
            tc.tile_pool(name="dram", bufs=1, space="DRAM") as dramp,
        ):
            # ---------------- constants ----------------
            ones = consts.tile([1, N], BF16)
            nc.vector.memset(ones, 1.0)
            ident64 = consts.tile([D, D], F32)
            make_identity(nc, ident64)

            # warmup collective: pays the CC barrier/firmware init cost up
            # front, overlapped with the projection phase
            ccw_in = dramp.tile([1, 8], F32, tag="ccwi")
            ccw_out = dramp.tile([1, 64], F32, tag="ccwo")
            nc.vector.memset(ccw_sb := consts.tile([1, 8], F32, name="ccw_sb"), 0.0)
            nc.gpsimd.dma_start(out=ccw_in[:, :], in_=ccw_sb[:, :])
            nc.gpsimd.collective_compute(
                "AllGather",
                mybir.AluOpType.bypass,
                replica_groups=[list(range(NCORES))],
                ins=[ccw_in.opt()],
                outs=[ccw_out.opt()],
            )

            wq_sb = consts.tile([128, KT_T, D], BF16)
            wk_sb = consts.tile([128, KT_T, D], BF16)
            wv_sb = consts.tile([128, KT_T, D], BF16)
            woc_sb = consts.tile([128, KT_T, D], BF16)
            for t_d, t_sb in ((wq_d, wq_sb), (wk_d, wk_sb), (wv_d, wv_sb),
                              (woc_d, woc_sb)):
                nc.sync.dma_start(out=t_sb,
                                  in_=t_d.ap().rearrange("(t p) d -> p t d", p=128))
            bias_sb = {}
            for nm, dd in (("bq", bq_d), ("bk", bk_d), ("bv", bv_d)):
                if dd is not None:
                    t = consts.tile([1, D], BF16, tag=f"bias_{nm}")
                    nc.sync.dma_start(out=t, in_=dd[:, :])
                    bias_sb[nm] = t
            if boc_d is not None:
                boc_sb = consts.tile([D, 1], F32)
                nc.sync.dma_start(out=boc_sb, in_=boc_d[:, :])

            # persistent intermediates
            QT_sb = pers.tile([D, N], BF16)    # Q^T/8 with bias folded
            KT_sb = pers.tile([D, N], BF16)
            VT_sb = pers.tile([D, N], F32)
            V_sb = pers.tile([128, nch, D + 1], BF16)  # [k, chunk, d | ones]
            SLICE = 1536
            NSL = N // SLICE
            # per-q-slice tiles so slice post-processing only depends on the
            # batches that actually wrote that slice (Tile dep granularity)
            hT_s = [pers.tile([D, SLICE], BF16, tag=f"hT{s}", name=f"hT{s}")
                    for s in range(NSL)]
            hTn_s = [pers.tile([D, SLICE], BF16, tag=f"hTn{s}", name=f"hTn{s}")
                     for s in range(NSL)]
            Zrow_s = [pers.tile([1, SLICE], F32, tag=f"Zr{s}", name=f"Zr{s}")
                      for s in range(NSL)]
            zbc_s = [pers.tile([D, SLICE], F32, tag=f"zbc{s}", name=f"zbc{s}")
                     for s in range(NSL)]

            # ---------------- projections ----------------
            def project_T(feat_d, w_sb, bias, dst):
                # dst[d, q] = (w.T @ featT)[d, q] (+ bias[d] via rank-1 mm)
                f_sb = featp.tile([128, KT_T, N], BF16, tag="feat")
                for h in range(4):
                    hsl = slice(h * (N // 4), (h + 1) * (N // 4))
                    for t in range(KT_T):
                        nc.sync.dma_start(
                            out=f_sb[:, t, hsl],
                            in_=feat_d.ap().rearrange("(t p) n -> t p n", p=128)[t, :, hsl],
                        )
                for qc in range(NQC):
                    qsl = slice(qc * 512, (qc + 1) * 512)
                    psum = ps_p.tile([128, 512], F32, tag="psp")
                    for t in range(KT_T):
                        nc.tensor.matmul(psum[0:D, :], w_sb[:, t, :],
                                         f_sb[:, t, qsl],
                                         start=(t == 0),
                                         stop=(t == KT_T - 1 and bias is None))
                    if bias is not None:
                        nc.tensor.matmul(psum[0:D, :], bias, ones[:, qsl],
                                         start=False, stop=True)
                    nc.any.tensor_copy(dst[:, qsl], psum[0:D, :])

            project_T(qfT_d, wq_sb, bias_sb.get("bq"), QT_sb)
            project_T(kfT_d, wk_sb, bias_sb.get("bk"), KT_sb)
            project_T(vfT_d, wv_sb, bias_sb.get("bv"), VT_sb)

            # V into batch-aligned k-chunks ([k, d] layout) via PE transposes
            nc.vector.memset(V_sb[:, :, D], 1.0)
            for j, (koff, klen) in enumerate(kchunk_list):
                pst = ps_p.tile([128, 512], F32, tag="psp")
                nc.tensor.transpose(pst[0:klen, 0:D], VT_sb[:, koff:koff + klen],
                                    ident64[:, :])
                nc.any.tensor_copy(V_sb[0:klen, j, 0:D], pst[0:klen, 0:D])

            # ------------- post-attention pipeline, per q-slice -------------
            def emit_slice(s):
                lo = s * SLICE
                # Z reciprocal: bounce q-major Z through DRAM into [128, w] so
                # reciprocal runs on 128 lanes, then back out flat (still
                # q-major) and broadcast-read across the 64 d-partitions with
                # contiguous row reads.
                w = SLICE // 128
                zb = dramp.tile([1, SLICE], F32, tag=f"zb{s}")
                nc.gpsimd.dma_start(out=zb[:, :], in_=Zrow_s[s][0:1, :])
                zres = pers.tile([128, w], F32, tag=f"zres{s}")
                nc.gpsimd.dma_start(out=zres[:, :],
                                    in_=zb[0:1, :].rearrange("p (a b) -> (p a) b", b=w))
                zrec = pers.tile([128, w], F32, tag=f"zrec{s}")
                nc.vector.reciprocal(zrec[:, :], zres[:, :])
                zr_d = dramp.tile([128, w], F32, tag=f"zrd{s}")
                nc.gpsimd.dma_start(out=zr_d[:, :], in_=zrec[:, :])
                zr_ap = zr_d[:, :]
                zbc_src = bass.AP(tensor=zr_ap.tensor, offset=zr_ap.offset,
                                  ap=[[0, D], [1, SLICE]])
                nc.gpsimd.dma_start(out=zbc_s[s][:, :], in_=zbc_src)
                # normalize hT
                nc.vector.tensor_mul(hTn_s[s][:, :], hT_s[s][:, :],
                                     zbc_s[s][:, :])
                # AllGather this slice of hT over heads
                ag_in = dramp.tile([D, SLICE], BF16, tag=f"agi{s}")
                nc.gpsimd.dma_start(out=ag_in[:, :], in_=hTn_s[s][:, :])
                ag_out = dramp.tile([OD, SLICE], BF16, tag=f"ago{s}")
                nc.gpsimd.collective_compute(
                    "AllGather",
                    mybir.AluOpType.bypass,
                    replica_groups=[list(range(NCORES))],
                    ins=[ag_in.opt()],
                    outs=[ag_out.opt()],
                )
                hTf_sb = featp.tile([128, KT_T, SLICE], BF16, tag="feat")
                for t in range(KT_T):
                    nc.gpsimd.dma_start(
                        out=hTf_sb[:, t, :],
                        in_=ag_out[:, :].rearrange("(t p) n -> t p n", p=128)[t],
                    )
                # output projection (column slice of Wo), transposed orientation
                for qc in range(SLICE // 512):
                    osl = slice(lo + qc * 512, lo + (qc + 1) * 512)
                    psum = ps_p.tile([128, 512], F32, tag="psp")
                    for t in range(KT_T):
                        nc.tensor.matmul(psum[0:D, :], woc_sb[:, t, :],
                                         hTf_sb[:, t, qc * 512:(qc + 1) * 512],
                                         start=(t == 0), stop=(t == KT_T - 1))
                    o_sb = outp.tile([D, 512], F32, tag="osb")
                    if boc_d is not None:
                        nc.scalar.activation(o_sb[:, :], psum[0:D, :],
                                             mybir.ActivationFunctionType.Identity,
                                             bias=boc_sb[:, 0:1])
                    else:
                        nc.any.tensor_copy(o_sb[:, :], psum[0:D, :])
                    nc.gpsimd.dma_start(out=out_d[:, osl], in_=o_sb[:, :])

            # ---------------- attention (block-diagonal) ----------------
            emitted = 0
            for bi, (q0, q1, k0, k1) in enumerate(batches):
                for (qoff, qw) in _chunks(q0, q1, 512):
                    qsl = slice(qoff, qoff + qw)
                    psum_h = ps_h.tile([D + 1, 512], F32, tag="psh")
                    idxs = batch_kchunks[bi]
                    for ii, j in enumerate(idxs):
                        koff, klen = kchunk_list[j]
                        ksl = slice(koff, koff + klen)
                        ps = ps_s.tile([128, 512], F32, tag="pss")
                        nc.tensor.matmul(ps[0:klen, 0:qw], KT_sb[:, ksl],
                                         QT_sb[:, qsl], start=True, stop=True)
                        pos = posp.tile([128, 512], BF16, tag="pos")
                        nc.sync.dma_start(out=pos[0:klen, 0:qw],
                                          in_=posT_d[ksl, qsl])
                        nc.vector.tensor_add(ps[0:klen, 0:qw], ps[0:klen, 0:qw],
                                             pos[0:klen, 0:qw])
                        expt = expp.tile([128, 512], BF16, tag="expt")
                        nc.scalar.activation(expt[0:klen, 0:qw], ps[0:klen, 0:qw],
                                             mybir.ActivationFunctionType.Exp)
                        nc.tensor.matmul(psum_h[:, 0:qw], V_sb[0:klen, j, :],
                                         expt[0:klen, 0:qw],
                                         start=(ii == 0), stop=(ii == len(idxs) - 1))
                    # copy h/Z out of PSUM, split at q-slice boundaries
                    seg = qoff
                    while seg < qoff + qw:
                        s = seg // SLICE
                        send = min(qoff + qw, (s + 1) * SLICE)
                        lsl = slice(seg - s * SLICE, send - s * SLICE)
                        psl = slice(seg - qoff, send - qoff)
                        nc.any.tensor_copy(hT_s[s][:, lsl], psum_h[0:D, psl])
                        nc.any.tensor_copy(Zrow_s[s][:, lsl],
                                           psum_h[D:D + 1, psl])
                        seg = send
                while emitted < NSL and (emitted + 1) * SLICE <= q1:
                    emit_slice(emitted)
                    emitted += 1
            assert emitted == NSL, (emitted, NSL, batches)

    nc.compile()
    return nc


def _kernel_numpy(q_feat, k_feat, v_feat, pos_enc, Wq, bq, Wk, bk, Wv, bv,
                  Wo, bo, q_batch, k_batch):
    """Host fallback (degenerate batch layouts only) + debugging aid."""
    Q = (q_feat @ Wq + bq).reshape(N, H, D).transpose(1, 0, 2)
    K = (k_feat @ Wk + bk).reshape(N, H, D).transpose(1, 0, 2)
    V = (v_feat @ Wv + bv).reshape(N, H, D).transpose(1, 0, 2)
    scores = np.einsum("hnd,hmd->hnm", Q, K) / SCALE + pos_enc
    mask = q_batch[:, None] != k_batch[None, :]
    scores = np.where(mask[None], np.float32(-1e9), scores)
    scores = scores - scores.max(-1, keepdims=True)
    e = np.exp(scores)
    probs = e / e.sum(-1, keepdims=True)
    h = np.einsum("hnm,hmd->hnd", probs, V)
    h = h.transpose(1, 0, 2).reshape(N, OD)
    return (h @ Wo + bo).astype(np.float32)


def kernel(q_feat, k_feat, v_feat, pos_enc, Wq, bq, Wk, bk, Wv, bv, Wo, bo,
           q_batch, k_batch):
    global LAST_RESULTS
    args = dict(q_feat=np.asarray(q_feat, np.float32),
                k_feat=np.asarray(k_feat, np.float32),
                v_feat=np.asarray(v_feat, np.float32),
                pos_enc=np.asarray(pos_enc, np.float32),
                Wq=np.asarray(Wq, np.float32), bq=np.asarray(bq, np.float32),
                Wk=np.asarray(Wk, np.float32), bk=np.asarray(bk, np.float32),
                Wv=np.asarray(Wv, np.float32), bv=np.asarray(bv, np.float32),
                Wo=np.asarray(Wo, np.float32), bo=np.asarray(bo, np.float32),
                q_batch=np.asarray(q_batch), k_batch=np.asarray(k_batch))

    batches, degenerate = _plan(args["q_batch"], args["k_batch"])
    if degenerate or not batches:
        return _kernel_numpy(**args)

    has_bq = bool(np.any(args["bq"]))
    has_bk = bool(np.any(args["bk"]))
    has_bv = bool(np.any(args["bv"]))
    has_bo = bool(np.any(args["bo"]))

    nc = _build(batches, has_bq, has_bk, has_bv, has_bo)

    # ---- host-side sharding / layout prep ----
    qfT = np.ascontiguousarray(args["q_feat"].T).astype(BF16_NP)
    kfT = np.ascontiguousarray(args["k_feat"].T).astype(BF16_NP)
    vfT = np.ascontiguousarray(args["v_feat"].T).astype(BF16_NP)

    in_maps = []
    for c in range(NCORES):
        hs = slice(c * D, (c + 1) * D)
        m = {
            "qfT": qfT, "kfT": kfT, "vfT": vfT,
            "posT": np.ascontiguousarray(
                args["pos_enc"][c].astype(BF16_NP).T),
            "wq": (args["Wq"][:, hs] / SCALE).astype(BF16_NP),
            "wk": args["Wk"][:, hs].astype(BF16_NP),
            "wv": args["Wv"][:, hs].astype(BF16_NP),
            "woc": np.ascontiguousarray(args["Wo"][:, hs]).astype(BF16_NP),
        }
        if has_bq:
            m["bq"] = (args["bq"][hs] / SCALE).astype(BF16_NP).reshape(1, D)
        if has_bk:
            m["bk"] = args["bk"][hs].astype(BF16_NP).reshape(1, D)
        if has_bv:
            m["bv"] = args["bv"][hs].astype(BF16_NP).reshape(1, D)
        if has_bo:
            m["boc"] = args["bo"][hs].astype(np.float32).reshape(D, 1)
        in_maps.append(m)

    res = run_bass_kernel_spmd(nc, in_maps, core_ids=list(range(NCORES)),
                               trace=TRACE)
    LAST_RESULTS = res
    out = np.empty((N, OD), np.float32)
    for c in range(NCORES):
        out[:, c * D:(c + 1) * D] = res.results[c]["out"].T
    return out


# revision 24
# speedup vs baseline: 1.2303x; 1.0077x over previous
"""Trainium2 Bass kernel for nn_MultiHeadAttention_3796751090171 (sparse_attention).

Head-parallel SPMD across 8 NeuronCores: core c computes head c's attention,
then the cores AllGather the (normalized) per-head context vectors and each
core computes a 64-column slice of the output projection — so no cross-core
reduction is ever needed (output = pure concatenation).

Math per head h (core c = h):
  Q = q_feat @ Wq[:, h*64:(h+1)*64] + bq_h          [N, 64]   (pre-scaled 1/8)
  K = k_feat @ Wk_h + bk_h                           [N, 64]
  V = v_feat @ Wv_h + bv_h                           [N, 64]
  S = Q @ K.T + pos_enc[h]   (block-diagonal only)   sparse [N, N]
  P = exp(S); Z = row sums (via ones-column in the V matmul)
  hT = (V|1).T @ expT ; hT /= Z
  -- AllGather hT over heads -> hTf [512, N] --
  outT_c = Wo[:, c*64:(c+1)*64].T @ hTf + bo_c       [64, N]
Host: out[:, c*64:(c+1)*64] = outT_c.T

Sparsity: q_batch/k_batch are SORTED, so the cross-batch mask is block-diagonal
over (q-range x k-range) batch blocks; we only compute those blocks and never
need elementwise masking (k-chunks are batch-aligned).

No max-subtraction in softmax: scores are O(10) so exp is safe in fp32; fully
masked blocks are simply never computed (prob contribution exactly 0, matching
the reference's exp(-1e9 - max) -> 0).
"""

import functools
import math

import numpy as np
import ml_dtypes

import concourse.bass as bass
import concourse.tile as tile
from concourse import bacc, mybir
from concourse.bass_utils import run_bass_kernel_spmd
from concourse.masks import make_identity

N = 3072
QD = 512
OD = 512
H = 8
D = 64
B = 8
NCORES = 8
SCALE = math.sqrt(D)

F32 = mybir.dt.float32
BF16 = mybir.dt.bfloat16
BF16_NP = ml_dtypes.bfloat16

# test.py can flip these to get a profile
TRACE = False
LAST_RESULTS = None


def _plan(q_batch, k_batch):
    """Batch block boundaries from the sorted batch-id vectors."""
    qb = np.asarray(q_batch).astype(np.int64)
    kb = np.asarray(k_batch).astype(np.int64)
    qbound = np.searchsorted(qb, np.arange(B + 1))
    kbound = np.searchsorted(kb, np.arange(B + 1))
    batches = []
    degenerate = False
    for b in range(B):
        q0, q1 = int(qbound[b]), int(qbound[b + 1])
        k0, k1 = int(kbound[b]), int(kbound[b + 1])
        if q1 > q0 and k1 > k0:
            batches.append((q0, q1, k0, k1))
        elif q1 > q0 and k1 == k0:
            # rows with no visible keys: reference gives uniform attention over
            # ALL keys; handled by numpy fallback (never happens in practice)
            degenerate = True
    return tuple(batches), degenerate


def _chunks(lo, hi, step):
    return [(o, min(step, hi - o)) for o in range(lo, hi, step)]


@functools.lru_cache(maxsize=8)
def _build(batches, has_bq, has_bk, has_bv, has_bo):
    nc = bacc.Bacc("TRN2", target_bir_lowering=False, debug=False,
                   num_devices=NCORES)

    # ---- DRAM parameters (per-core values supplied via in_maps) ----
    qfT_d = nc.dram_tensor("qfT", [QD, N], BF16, kind="ExternalInput")
    kfT_d = nc.dram_tensor("kfT", [QD, N], BF16, kind="ExternalInput")
    vfT_d = nc.dram_tensor("vfT", [QD, N], BF16, kind="ExternalInput")
    posT_d = nc.dram_tensor("posT", [N, N], BF16, kind="ExternalInput")
    wq_d = nc.dram_tensor("wq", [QD, D], BF16, kind="ExternalInput")
    wk_d = nc.dram_tensor("wk", [QD, D], BF16, kind="ExternalInput")
    wv_d = nc.dram_tensor("wv", [QD, D], BF16, kind="ExternalInput")
    woc_d = nc.dram_tensor("woc", [OD, D], BF16, kind="ExternalInput")
    bq_d = nc.dram_tensor("bq", [1, D], BF16, kind="ExternalInput") if has_bq else None
    bk_d = nc.dram_tensor("bk", [1, D], BF16, kind="ExternalInput") if has_bk else None
    bv_d = nc.dram_tensor("bv", [1, D], BF16, kind="ExternalInput") if has_bv else None
    boc_d = nc.dram_tensor("boc", [D, 1], F32, kind="ExternalInput") if has_bo else None
    out_d = nc.dram_tensor("out", [D, N], F32, kind="ExternalOutput")

    # global k-chunk list, aligned to batch boundaries (never crosses one)
    kchunk_list = []   # (koff, klen)
    batch_kchunks = []  # per batch: list of global chunk indices
    for (q0, q1, k0, k1) in batches:
        idxs = []
        for (koff, klen) in _chunks(k0, k1, 128):
            idxs.append(len(kchunk_list))
            kchunk_list.append((koff, klen))
        batch_kchunks.append(idxs)
    nch = len(kchunk_list)

    KT_T = 4  # 512 contraction split in 4 k-tiles of 128
    NQC = N // 512
    NZC = N // 128  # 24 z chunks

    with tile.TileContext(nc) as tc:
        with (
            tc.tile_pool(name="consts", bufs=1) as consts,
            tc.tile_pool(name="feat", bufs=2) as featp,
            tc.tile_pool(name="persist", bufs=1) as pers,
            tc.tile_pool(name="pos", bufs=10) as posp,
            tc.tile_pool(name="expp", bufs=6) as expp,
            tc.tile_pool(name="outp", bufs=3) as outp,
            tc.tile_pool(name="ps_s", bufs=3, space="PSUM") as ps_s,
            tc.tile_pool(name="ps_h", bufs=2, space="PSUM") as ps_h,
            tc.tile_pool(name="ps_p", bufs=3, space="PSUM") as ps_p,
            tc.tile_pool(name="dram", bufs=1, space="DRAM") as dramp,
        ):
            # ---------------- constants ----------------
            ones = consts.tile([1, N], BF16)
            nc.vector.memset(ones, 1.0)
            ident64 = consts.tile([D, D], F32)
            make_identity(nc, ident64)

            # warmup collective: pays the CC barrier/firmware init cost up
            # front, overlapped with the projection phase
            ccw_in = dramp.tile([1, 8], F32, tag="ccwi")
            ccw_out = dramp.tile([1, 64], F32, tag="ccwo")
            nc.vector.memset(ccw_sb := consts.tile([1, 8], F32, name="ccw_sb"), 0.0)
            nc.gpsimd.dma_start(out=ccw_in[:, :], in_=ccw_sb[:, :])
            nc.gpsimd.collective_compute(
                "AllGather",
                mybir.AluOpType.bypass,
                replica_groups=[list(range(NCORES))],
                ins=[ccw_in.opt()],
                outs=[ccw_out.opt()],
            )

            wq_sb = consts.tile([128, KT_T, D], BF16)
            wk_sb = consts.tile([128, KT_T, D], BF16)
            wv_sb = consts.tile([128, KT_T, D], BF16)
            woc_sb = consts.tile([128, KT_T, D], BF16)
            for t_d, t_sb in ((wq_d, wq_sb), (wk_d, wk_sb), (wv_d, wv_sb),
                              (woc_d, woc_sb)):
                nc.sync.dma_start(out=t_sb,
                                  in_=t_d.ap().rearrange("(t p) d -> p t d", p=128))
            bias_sb = {}
            for nm, dd in (("bq", bq_d), ("bk", bk_d), ("bv", bv_d)):
                if dd is not None:
                    t = consts.tile([1, D], BF16, tag=f"bias_{nm}")
                    nc.sync.dma_start(out=t, in_=dd[:, :])
                    bias_sb[nm] = t
            if boc_d is not None:
                boc_sb = consts.tile([D, 1], F32)
                nc.sync.dma_start(out=boc_sb, in_=boc_d[:, :])

            # persistent intermediates
            QT_sb = pers.tile([D, N], BF16)    # Q^T/8 with bias folded
            KT_sb = pers.tile([D, N], BF16)
            VT_sb = pers.tile([D, N], F32)
            V_sb = pers.tile([128, nch, D + 1], BF16)  # [k, chunk, d | ones]
            SLICE = 1536
            NSL = N // SLICE
            # per-q-slice tiles so slice post-processing only depends on the
            # batches that actually wrote that slice (Tile dep granularity)
            hT_s = [pers.tile([D, SLICE], BF16, tag=f"hT{s}", name=f"hT{s}")
                    for s in range(NSL)]
            hTn_s = [pers.tile([D, SLICE], BF16, tag=f"hTn{s}", name=f"hTn{s}")
                     for s in range(NSL)]
            Zrow_s = [pers.tile([1, SLICE], F32, tag=f"Zr{s}", name=f"Zr{s}")
                      for s in range(NSL)]
            zbc_s = [pers.tile([D, SLICE], F32, tag=f"zbc{s}", name=f"zbc{s}")
                     for s in range(NSL)]

            # ---------------- projections ----------------
            def project_T(feat_d, w_sb, bias, dst):
                # dst[d, q] = (w.T @ featT)[d, q] (+ bias[d] via rank-1 mm)
                f_sb = featp.tile([128, KT_T, N], BF16, tag="feat")
                for h in range(4):
                    hsl = slice(h * (N // 4), (h + 1) * (N // 4))
                    for t in range(KT_T):
                        nc.sync.dma_start(
                            out=f_sb[:, t, hsl],
                            in_=feat_d.ap().rearrange("(t p) n -> t p n", p=128)[t, :, hsl],
                        )
                for qc in range(NQC):
                    qsl = slice(qc * 512, (qc + 1) * 512)
                    psum = ps_p.tile([128, 512], F32, tag="psp")
                    for t in range(KT_T):
                        nc.tensor.matmul(psum[0:D, :], w_sb[:, t, :],
                                         f_sb[:, t, qsl],
                                         start=(t == 0),
                                         stop=(t == KT_T - 1 and bias is None))
                    if bias is not None:
                        nc.tensor.matmul(psum[0:D, :], bias, ones[:, qsl],
                                         start=False, stop=True)
                    nc.vector.tensor_copy(dst[:, qsl], psum[0:D, :])

            project_T(qfT_d, wq_sb, bias_sb.get("bq"), QT_sb)
            project_T(kfT_d, wk_sb, bias_sb.get("bk"), KT_sb)
            project_T(vfT_d, wv_sb, bias_sb.get("bv"), VT_sb)

            # V into batch-aligned k-chunks ([k, d] layout) via PE transposes
            nc.vector.memset(V_sb[:, :, D], 1.0)
            for j, (koff, klen) in enumerate(kchunk_list):
                pst = ps_p.tile([128, 512], F32, tag="psp")
                nc.tensor.transpose(pst[0:klen, 0:D], VT_sb[:, koff:koff + klen],
                                    ident64[:, :])
                nc.scalar.copy(V_sb[0:klen, j, 0:D], pst[0:klen, 0:D])

            # ------------- post-attention pipeline, per q-slice -------------
            def emit_slice(s):
                lo = s * SLICE
                # Z reciprocal: bounce q-major Z through DRAM into [128, w] so
                # reciprocal runs on 128 lanes, then back out flat (still
                # q-major) and broadcast-read across the 64 d-partitions with
                # contiguous row reads.
                w = SLICE // 128
                zb = dramp.tile([1, SLICE], F32, tag=f"zb{s}")
                nc.gpsimd.dma_start(out=zb[:, :], in_=Zrow_s[s][0:1, :])
                zres = pers.tile([128, w], F32, tag=f"zres{s}")
                nc.gpsimd.dma_start(out=zres[:, :],
                                    in_=zb[0:1, :].rearrange("p (a b) -> (p a) b", b=w))
                zrec = pers.tile([128, w], F32, tag=f"zrec{s}")
                nc.vector.reciprocal(zrec[:, :], zres[:, :])
                zr_d = dramp.tile([128, w], F32, tag=f"zrd{s}")
                nc.gpsimd.dma_start(out=zr_d[:, :], in_=zrec[:, :])
                zr_ap = zr_d[:, :]
                zbc_src = bass.AP(tensor=zr_ap.tensor, offset=zr_ap.offset,
                                  ap=[[0, D], [1, SLICE]])
                nc.gpsimd.dma_start(out=zbc_s[s][:, :], in_=zbc_src)
                # normalize hT
                nc.vector.tensor_mul(hTn_s[s][:, :], hT_s[s][:, :],
                                     zbc_s[s][:, :])
                # AllGather this slice of hT over heads
                ag_in = dramp.tile([D, SLICE], BF16, tag=f"agi{s}")
                nc.gpsimd.dma_start(out=ag_in[:, :], in_=hTn_s[s][:, :])
                ag_out = dramp.tile([OD, SLICE], BF16, tag=f"ago{s}")
                nc.gpsimd.collective_compute(
                    "AllGather",
                    mybir.AluOpType.bypass,
                    replica_groups=[list(range(NCORES))],
                    ins=[ag_in.opt()],
                    outs=[ag_out.opt()],
                )
                hTf_sb = featp.tile([128, KT_T, SLICE], BF16, tag="feat")
                for t in range(KT_T):
                    nc.gpsimd.dma_start(
                        out=hTf_sb[:, t, :],
                        in_=ag_out[:, :].rearrange("(t p) n -> t p n", p=128)[t],
                    )
                # output projection (column slice of Wo), transposed orientation
                for qc in range(SLICE // 512):
                    osl = slice(lo + qc * 512, lo + (qc + 1) * 512)
                    psum = ps_p.tile([128, 512], F32, tag="psp")
                    for t in range(KT_T):
                        nc.tensor.matmul(psum[0:D, :], woc_sb[:, t, :],
                                         hTf_sb[:, t, qc * 512:(qc + 1) * 512],
                                         start=(t == 0), stop=(t == KT_T - 1))
                    o_sb = outp.tile([D, 512], F32, tag="osb")
                    if boc_d is not None:
                        nc.scalar.activation(o_sb[:, :], psum[0:D, :],
                                             mybir.ActivationFunctionType.Identity,
                                             bias=boc_sb[:, 0:1])
                    else:
                        nc.vector.tensor_copy(o_sb[:, :], psum[0:D, :])
                    nc.gpsimd.dma_start(out=out_d[:, osl], in_=o_sb[:, :])

            # ---------------- attention (block-diagonal) ----------------
            emitted = 0
            for bi, (q0, q1, k0, k1) in enumerate(batches):
                for (qoff, qw) in _chunks(q0, q1, 512):
                    qsl = slice(qoff, qoff + qw)
                    psum_h = ps_h.tile([D + 1, 512], F32, tag="psh")
                    idxs = batch_kchunks[bi]
                    for ii, j in enumerate(idxs):
                        koff, klen = kchunk_list[j]
                        ksl = slice(koff, koff + klen)
                        ps = ps_s.tile([128, 512], F32, tag="pss")
                        nc.tensor.matmul(ps[0:klen, 0:qw], KT_sb[:, ksl],
                                         QT_sb[:, qsl], start=True, stop=True)
                        pos = posp.tile([128, 512], BF16, tag="pos")
                        nc.sync.dma_start(out=pos[0:klen, 0:qw],
                                          in_=posT_d[ksl, qsl])
                        nc.vector.tensor_add(ps[0:klen, 0:qw], ps[0:klen, 0:qw],
                                             pos[0:klen, 0:qw])
                        expt = expp.tile([128, 512], BF16, tag="expt")
                        nc.scalar.activation(expt[0:klen, 0:qw], ps[0:klen, 0:qw],
                                             mybir.ActivationFunctionType.Exp)
                        nc.tensor.matmul(psum_h[:, 0:qw], V_sb[0:klen, j, :],
                                         expt[0:klen, 0:qw],
                                         start=(ii == 0), stop=(ii == len(idxs) - 1))
                    # copy h/Z out of PSUM, split at q-slice boundaries
                    seg = qoff
                    while seg < qoff + qw:
                        s = seg // SLICE
                        send = min(qoff + qw, (s + 1) * SLICE)
                        lsl = slice(seg - s * SLICE, send - s * SLICE)
                        psl = slice(seg - qoff, send - qoff)
                        nc.scalar.copy(hT_s[s][:, lsl], psum_h[0:D, psl])
                        nc.vector.tensor_copy(Zrow_s[s][:, lsl],
                                              psum_h[D:D + 1, psl])
                        seg = send
                while emitted < NSL and (emitted + 1) * SLICE <= q1:
                    emit_slice(emitted)
                    emitted += 1
            assert emitted == NSL, (emitted, NSL, batches)

    nc.compile()
    return nc


def _kernel_numpy(q_feat, k_feat, v_feat, pos_enc, Wq, bq, Wk, bk, Wv, bv,
                  Wo, bo, q_batch, k_batch):
    """Host fallback (degenerate batch layouts only) + debugging aid."""
    Q = (q_feat @ Wq + bq).reshape(N, H, D).transpose(1, 0, 2)
    K = (k_feat @ Wk + bk).reshape(N, H, D).transpose(1, 0, 2)
    V = (v_feat @ Wv + bv).reshape(N, H, D).transpose(1, 0, 2)
    scores = np.einsum("hnd,hmd->hnm", Q, K) / SCALE + pos_enc
    mask = q_batch[:, None] != k_batch[None, :]
    scores = np.where(mask[None], np.float32(-1e9), scores)
    scores = scores - scores.max(-1, keepdims=True)
    e = np.exp(scores)
    probs = e / e.sum(-1, keepdims=True)
    h = np.einsum("hnm,hmd->hnd", probs, V)
    h = h.transpose(1, 0, 2).reshape(N, OD)
    return (h @ Wo + bo).astype(np.float32)


def kernel(q_feat, k_feat, v_feat, pos_enc, Wq, bq, Wk, bk, Wv, bv, Wo, bo,
           q_batch, k_batch):
    global LAST_RESULTS
    args = dict(q_feat=np.asarray(q_feat, np.float32),
                k_feat=np.asarray(k_feat, np.float32),
                v_feat=np.asarray(v_feat, np.float32),
                pos_enc=np.asarray(pos_enc, np.float32),
                Wq=np.asarray(Wq, np.float32), bq=np.asarray(bq, np.float32),
                Wk=np.asarray(Wk, np.float32), bk=np.asarray(bk, np.float32),
                Wv=np.asarray(Wv, np.float32), bv=np.asarray(bv, np.float32),
                Wo=np.asarray(Wo, np.float32), bo=np.asarray(bo, np.float32),
                q_batch=np.asarray(q_batch), k_batch=np.asarray(k_batch))

    batches, degenerate = _plan(args["q_batch"], args["k_batch"])
    if degenerate or not batches:
        return _kernel_numpy(**args)

    has_bq = bool(np.any(args["bq"]))
    has_bk = bool(np.any(args["bk"]))
    has_bv = bool(np.any(args["bv"]))
    has_bo = bool(np.any(args["bo"]))

    nc = _build(batches, has_bq, has_bk, has_bv, has_bo)

    # ---- host-side sharding / layout prep ----
    qfT = np.ascontiguousarray(args["q_feat"].T).astype(BF16_NP)
    kfT = np.ascontiguousarray(args["k_feat"].T).astype(BF16_NP)
    vfT = np.ascontiguousarray(args["v_feat"].T).astype(BF16_NP)

    in_maps = []
    for c in range(NCORES):
        hs = slice(c * D, (c + 1) * D)
        m = {
            "qfT": qfT, "kfT": kfT, "vfT": vfT,
            "posT": np.ascontiguousarray(
                args["pos_enc"][c].astype(BF16_NP).T),
            "wq": (args["Wq"][:, hs] / SCALE).astype(BF16_NP),
            "wk": args["Wk"][:, hs].astype(BF16_NP),
            "wv": args["Wv"][:, hs].astype(BF16_NP),
            "woc": np.ascontiguousarray(args["Wo"][:, hs]).astype(BF16_NP),
        }
        if has_bq:
            m["bq"] = (args["bq"][hs] / SCALE).astype(BF16_NP).reshape(1, D)
        if has_bk:
            m["bk"] = args["bk"][hs].astype(BF16_NP).reshape(1, D)
        if has_bv:
            m["bv"] = args["bv"][hs].astype(BF16_NP).reshape(1, D)
        if has_bo:
            m["boc"] = args["bo"][hs].astype(np.float32).reshape(D, 1)
        in_maps.append(m)

    res = run_bass_kernel_spmd(nc, in_maps, core_ids=list(range(NCORES)),
                               trace=TRACE)
    LAST_RESULTS = res
    out = np.empty((N, OD), np.float32)
    for c in range(NCORES):
        out[:, c * D:(c + 1) * D] = res.results[c]["out"].T
    return out
